# revision 1
# baseline (speedup 1.0000x reference)
"""Cross-attention kernel for Trainium2, 8-core head-sharded (tensor parallel).

Problem: x[2,2048,1024], context[2,2048,768], Wq[1024,1024], Wkv[768,2048],
Wo[1024,1024]; H=16 heads, Dh=64; out = softmax(q k^T / 8) v @ Wo.

Sharding: 2 heads per core (128 q/k/v columns). Each core computes its
heads' attention output projected through its 128-row slice of Wo,
producing a full-shape partial; host sums the 8 partials.

Per-core dataflow (all matmuls bf16 with fp32 PSUM accumulation):
  qT[128,4096]  = Wq_slice^T @ x^T        (lhsT = Wq_slice, rhs = xT)
  kT[128,4096]  = Wk_slice^T @ ctx^T
  v[4096,130]   = ctx @ Wv_slice, +ones columns (flash denominator trick)
  per (batch b, head h, n-tile of 512 queries):
    S^T[m,n]    = kT_h^T-slice matmuls  (K=64)  -> PSUM [128, 4xn512]
    expST       = Exp(S^T * 0.125)  on ScalarE -> SBUF bf16
    OT[65,512] += v_aug_h^T @ expST  (K=128, accumulate over 16 m-tiles)
                  row 64 = softmax denominator
    OT_norm     = OT[0:64] * bcast(1/denom)   (DVE + gpsimd broadcast)
  out[n,1024]   = OTcomb^T @ Wo_slice  -> partial output, DMA to DRAM
"""

import numpy as np
import ml_dtypes

import concourse.bass as bass
import concourse.mybir as mybir
import concourse.tile as tile
from concourse import bacc
from concourse.bass_utils import run_bass_kernel_spmd

BF16 = mybir.dt.bfloat16
F32 = mybir.dt.float32
NPBF16 = ml_dtypes.bfloat16

B, N, C = 2, 2048, 1024
M = 2048          # context length
CTX = 768
H = 16
DH = 64
NCORES = 8
HEADS_PER_CORE = H // NCORES          # 2
HC = HEADS_PER_CORE * DH              # 128 columns per core
ROWS = B * N                          # 4096 flattened rows
SCALE = DH ** -0.5                    # 0.125

KQ = C // 128                         # 8 k-tiles for q projection
KC = CTX // 128                       # 6 k-tiles for kv projections
NT = 512                              # query-tile width (free dim)
MT = 128                              # m-tile (context block = partitions)
N_NT = N // NT                        # 4 query tiles per batch
N_MT = M // MT                        # 16 m-tiles per batch
MG = 2                                # m-tiles per exp group ([128,1024] ACT calls)

_CACHE = {}


PACK_QK = False


def build_kernel(repeat=1, pack_qk=None):
    """Build and compile the per-core Bass module (same program, all cores).

    repeat>1 repeats the whole body (bench-only: marginal time between two
    repeat counts isolates steady-state per-iteration HW time from the
    axon dispatch overhead)."""
    if pack_qk is None:
        pack_qk = PACK_QK
    nc = bacc.Bacc(None)

    xT_d = nc.dram_tensor("xT", [C, ROWS], BF16, kind="ExternalInput")
    cT_d = nc.dram_tensor("ctxT", [CTX, ROWS], BF16, kind="ExternalInput")
    wq_d = nc.dram_tensor("wq", [C, HC], BF16, kind="ExternalInput")
    wk_d = nc.dram_tensor("wk", [CTX, HC], BF16, kind="ExternalInput")
    wv_d = nc.dram_tensor("wv", [CTX, HC], BF16, kind="ExternalInput")
    wo_d = nc.dram_tensor("wo", [HC, C], BF16, kind="ExternalInput")
    out_d = nc.dram_tensor("out", [ROWS, C], F32, kind="ExternalOutput")

    xT_t = xT_d.rearrange("(t p) n -> t p n", p=128)     # [8,128,4096]
    cT_t = cT_d.rearrange("(t p) n -> t p n", p=128)     # [6,128,4096]

    with tile.TileContext(nc) as tc:
        with (
            tc.tile_pool(name="const", bufs=1) as const,
            tc.tile_pool(name="ctx_res", bufs=1) as ctx_res,
            tc.tile_pool(name="act_res", bufs=1) as act_res,
            tc.tile_pool(name="xstream", bufs=4) as xstream,
            tc.tile_pool(name="expp", bufs=6) as expp,
            tc.tile_pool(name="otcomb", bufs=2) as otcomb_p,
            tc.tile_pool(name="nrm", bufs=4) as nrm,
            tc.tile_pool(name="ostage", bufs=3) as ostage,
            tc.tile_pool(name="pst", bufs=2, space="PSUM") as pst,
            tc.tile_pool(name="pot", bufs=3, space="PSUM") as pot,
            tc.tile_pool(name="pq", bufs=1, space="PSUM") as pq,
        ):
          for _rep in range(repeat):
              # ---- weights (wq/wk now; wv/wo deferred past the first
              # x/ctx loads so the critical-path DMAs lead the queue) ----
              wq_sb = const.tile([128, KQ, HC], BF16, tag="wq")
              nc.sync.dma_start(out=wq_sb, in_=wq_d.rearrange("(t p) m -> p t m", p=128))
              wk_sb = const.tile([128, KC, HC], BF16, tag="wk")
              nc.sync.dma_start(out=wk_sb, in_=wk_d.rearrange("(t p) m -> p t m", p=128))
              wv_sb = const.tile([128, KC, HC], BF16, tag="wv")
              wo_sb = const.tile([128, C], BF16, tag="wo")

              # resident context; quarters loaded just-in-time in the batch loop
              ctx_sb = ctx_res.tile([128, KC, ROWS], BF16, tag="ctx")
              cT_pm = cT_d.rearrange("(t p) n -> p t n", p=128)   # [128,6,4096]
              xT_pm = xT_d.rearrange("(t p) n -> p t n", p=128)   # [128,8,4096]

              kT_sb = act_res.tile([128, ROWS], BF16, tag="kT")
              vago = act_res.tile([128, ROWS // 128, 130], BF16, tag="vaug")
              nc.vector.memset(vago[:, :, 64], 1.0)
              nc.vector.memset(vago[:, :, 129], 1.0)
              qT_sb = act_res.tile([128, ROWS], BF16, tag="qT")

              def qt_chunk(n):
                  """Project qT columns [n*512, (n+1)*512)."""
                  xs = xstream.tile([128, KQ, NT], BF16, tag="xs")
                  nc.sync.dma_start(out=xs, in_=xT_pm[:, :, bass.ts(n, NT)])
                  psq = pq.tile([128, NT], F32, tag="pq")
                  for t in range(KQ):
                      nc.tensor.matmul(psq, wq_sb[:, t, :], xs[:, t, :],
                                       start=(t == 0), stop=(t == KQ - 1))
                  nc.vector.tensor_copy(qT_sb[:, bass.ts(n, NT)], psq)

              def ctx_quarter(n):
                  nc.sync.dma_start(out=ctx_sb[:, :, bass.ts(n, NT)],
                                    in_=cT_pm[:, :, bass.ts(n, NT)])

              def kv_chunk(n):
                  """Project kT cols + 4 v tiles for m-quarter n (global)."""
                  ps = pq.tile([128, NT], F32, tag="pq")
                  for t in range(KC):
                      nc.tensor.matmul(ps, wk_sb[:, t, :],
                                       ctx_sb[:, t, bass.ts(n, NT)],
                                       start=(t == 0), stop=(t == KC - 1))
                  nc.vector.tensor_copy(kT_sb[:, bass.ts(n, NT)], ps)
                  for mm in range(NT // 128):
                      m = n * 4 + mm
                      psv = pq.tile([128, NT], F32, tag="pq")
                      for t in range(KC):
                          nc.tensor.matmul(psv[:, 0:HC],
                                           ctx_sb[:, t, bass.ts(m, 128)],
                                           wv_sb[:, t, :],
                                           start=(t == 0), stop=(t == KC - 1))
                      nc.vector.tensor_copy(vago[:, m, 0:64], psv[:, 0:64])
                      nc.vector.tensor_copy(vago[:, m, 65:129], psv[:, 64:128])

              def attn_groups(b, nt, ot_A, ot_B, groups, otc=None):
                  nsl = bass.ds(b * N + nt * NT, NT)
                  for g in groups:
                      if pack_qk:
                          # fill both heads' S^T tiles first: row-tiled pairs
                          # run concurrently in the PE array (rows 0:64, 64:128)
                          st_pa = pst.tile([128, MG * NT], F32, tag="st")
                          st_pb = pst.tile([128, MG * NT], F32, tag="st")
                          sts = [st_pa, st_pb]
                          for j in range(MG):
                              mt = g * MG + j
                              msl = bass.ds(b * M + mt * MT, MT)
                              for h in range(2):
                                  hd = bass.ds(h * DH, DH)
                                  nc.tensor.matmul(sts[h][:, bass.ts(j, NT)],
                                                   kT_sb[hd, msl], qT_sb[hd, nsl],
                                                   start=True, stop=True,
                                                   tile_position=(h * DH, 0))
                      for h, ot_ps in enumerate((ot_A, ot_B)):
                          hd = bass.ds(h * DH, DH)
                          vsl = bass.ds(h * 65, 65)
                          if pack_qk:
                              st_ps = sts[h]
                          else:
                              st_ps = pst.tile([128, MG * NT], F32, tag="st")
                          exp_sb = expp.tile([128, MG * NT], BF16, tag="exp")
                          if not pack_qk:
                              for j in range(MG):
                                  mt = g * MG + j
                                  msl = bass.ds(b * M + mt * MT, MT)
                                  nc.tensor.matmul(st_ps[:, bass.ts(j, NT)],
                                                   kT_sb[hd, msl], qT_sb[hd, nsl],
                                                   start=True, stop=True)
                          nc.scalar.activation(
                              exp_sb, st_ps,
                              mybir.ActivationFunctionType.Exp, scale=SCALE)
                          for j in range(MG):
                              mt = g * MG + j
                              nc.tensor.matmul(
                                  ot_ps,
                                  vago[:, (b * M) // 128 + mt, vsl],
                                  exp_sb[:, bass.ts(j, NT)],
                                  start=(mt == 0), stop=(mt == N_MT - 1))
                          if otc is not None and h == 0 and g == N_MT // MG - 1:
                              normalize_head(0, ot_A, otc)

              def normalize_head(h, ot_ps, otc):
                  """softmax-normalize one head's O^T into otc (bf16)."""
                  rec = nrm.tile([1, NT], F32, tag="rec")
                  nc.vector.reciprocal(rec, ot_ps[64:65, :])
                  bc = nrm.tile([64, NT], F32, tag="bc")
                  nc.gpsimd.partition_broadcast(bc, rec)
                  if h == 0:
                      nc.vector.tensor_mul(otc[0:64, :], ot_ps[0:64, :], bc)
                  else:
                      otn = nrm.tile([64, NT], BF16, tag="otn")
                      nc.vector.tensor_mul(otn, ot_ps[0:64, :], bc)
                      # partition shift 0:64 -> 64:128 via SBUF->SBUF DMA
                      nc.sync.dma_start(out=otc[64:128, :], in_=otn)

              def final_out(b, nt, otc):
                  """out[rows, :] = otc^T @ Wo  (PSUM slots shared with pot)."""
                  for s in range(NT // 128):
                      ost = ostage.tile([128, C], F32, tag="ost")
                      for cpart in range(C // NT):
                          fp = pot.tile([128, NT], F32, tag="ot")
                          nc.tensor.matmul(fp, otc[:, bass.ts(s, 128)],
                                           wo_sb[:, bass.ts(cpart, NT)],
                                           start=True, stop=True)
                          nc.vector.tensor_copy(ost[:, bass.ts(cpart, NT)], fp)
                      nc.sync.dma_start(
                          out=out_d[bass.ds(b * N + nt * NT + s * 128, 128), :],
                          in_=ost)

              pending = None      # (b, nt, otc) awaiting final projection
              # batch 0: projections interleaved with its first tile
              qt_chunk(0)
              nc.sync.dma_start(
                  out=wv_sb, in_=wv_d.rearrange("(t p) m -> p t m", p=128))
              ot_A = pot.tile([65, NT], F32, tag="ot")
              ot_B = pot.tile([65, NT], F32, tag="ot")
              otc = otcomb_p.tile([128, NT], BF16, tag="otc")
              for c in range(4):
                  ctx_quarter(c)
                  if c == 0:
                      nc.sync.dma_start(out=wo_sb, in_=wo_d[:])
                  kv_chunk(c)
                  attn_groups(0, 0, ot_A, ot_B, (2 * c, 2 * c + 1), otc=otc)
                  if c >= 1:
                      qt_chunk(c)
              normalize_head(1, ot_B, otc)
              pending = (0, 0, otc)
              # batch 0 tiles 1-3, with batch 1's projections hidden under
              # their ACT-bound attention (one chunk pair per tile)
              for nt in range(1, N_NT):
                  ot_A = pot.tile([65, NT], F32, tag="ot")
                  ot_B = pot.tile([65, NT], F32, tag="ot")
                  otc = otcomb_p.tile([128, NT], BF16, tag="otc")
                  attn_groups(0, nt, ot_A, ot_B, (0, 1))
                  final_out(*pending)
                  c = nt - 1            # b1 chunks 0..2 under b0 tiles 1..3
                  ctx_quarter(4 + c)
                  kv_chunk(4 + c)
                  attn_groups(0, nt, ot_A, ot_B, (2, 3))
                  qt_chunk(N_NT + c)
                  attn_groups(0, nt, ot_A, ot_B, range(4, N_MT // MG), otc=otc)
                  normalize_head(1, ot_B, otc)
                  pending = (0, nt, otc)
              # batch 1
              for nt in range(N_NT):
                  ot_A = pot.tile([65, NT], F32, tag="ot")
                  ot_B = pot.tile([65, NT], F32, tag="ot")
                  otc = otcomb_p.tile([128, NT], BF16, tag="otc")
                  attn_groups(1, nt, ot_A, ot_B, (0, 1))
                  final_out(*pending)
                  if nt == 0:
                      ctx_quarter(7)
                      kv_chunk(7)
                      attn_groups(1, 0, ot_A, ot_B, (2, 3, 4, 5))
                      qt_chunk(N_NT + 3)
                      attn_groups(1, 0, ot_A, ot_B, (6, 7), otc=otc)
                  else:
                      attn_groups(1, nt, ot_A, ot_B, range(2, N_MT // MG), otc=otc)
                  normalize_head(1, ot_B, otc)
                  pending = (1, nt, otc)
              final_out(*pending)

    nc.compile()
    return nc


def _shard_inputs(x, context, Wq, Wkv, Wo):
    xf = np.ascontiguousarray(x.reshape(ROWS, C).T).astype(NPBF16)
    cf = np.ascontiguousarray(context.reshape(ROWS, CTX).T).astype(NPBF16)
    in_maps = []
    for c in range(NCORES):
        hc = slice(HC * c, HC * (c + 1))
        in_maps.append({
            "xT": xf,
            "ctxT": cf,
            "wq": np.ascontiguousarray(Wq[:, hc]).astype(NPBF16),
            "wk": np.ascontiguousarray(Wkv[:, hc]).astype(NPBF16),
            "wv": np.ascontiguousarray(Wkv[:, C + HC * c:C + HC * (c + 1)]).astype(NPBF16),
            "wo": np.ascontiguousarray(Wo[hc, :]).astype(NPBF16),
        })
    return in_maps


def get_nc():
    if "nc" not in _CACHE:
        _CACHE["nc"] = build_kernel()
    return _CACHE["nc"]


def run_cores(in_maps, **kw):
    nc = get_nc()
    return run_bass_kernel_spmd(nc, in_maps, list(range(NCORES)), **kw)


def kernel(x, context, Wq, Wkv, Wo):
    in_maps = _shard_inputs(
        np.asarray(x, np.float32), np.asarray(context, np.float32),
        np.asarray(Wq, np.float32), np.asarray(Wkv, np.float32),
        np.asarray(Wo, np.float32))
    res = run_cores(in_maps)
    acc = res.results[0]["out"].astype(np.float32)
    for i in range(1, NCORES):
        acc = acc + res.results[i]["out"]
    return acc.reshape(B, N, C)



# revision 32
# speedup vs baseline: 39234.6624x; 39234.6624x over previous
"""Cross-attention kernel for Trainium2, 8-core head-sharded (tensor parallel).

Problem: x[2,2048,1024], context[2,2048,768], Wq[1024,1024], Wkv[768,2048],
Wo[1024,1024]; H=16 heads, Dh=64; out = softmax(q k^T / 8) v @ Wo.

Sharding: 2 heads per core (128 q/k/v columns). Each core computes its
heads' attention output projected through its 128-row slice of Wo,
producing a full-shape partial; host sums the 8 partials.

Per-core dataflow (all matmuls bf16 with fp32 PSUM accumulation):
  qT[128,4096]  = Wq_slice^T @ x^T        (lhsT = Wq_slice, rhs = xT)
  kT[128,4096]  = Wk_slice^T @ ctx^T
  v[4096,130]   = ctx @ Wv_slice, +ones columns (flash denominator trick)
  per (batch b, n-tile of 512 queries, m-tile of 128 keys):
    S^T[128,1024]  = both heads' K=64 matmuls packed concurrently into
                     the PE array via tile_position row groups (0/64)
    exp[128,1024]  = Exp(S^T * 0.125) on ScalarE, one call per m-tile
    OT_h[65,512]  += v_aug_h^T @ exp_h  (K=128, accum over 16 m-tiles)
                     row 64 = softmax denominator
  normalize (per n-tile): ScalarE copies evacuate OT from PSUM (frees the
    AV accumulator banks fast, off the DVE FIFO); the [1,1024] denominator
    row is reciprocated lane-parallel via the DVE 32x32-transpose trick;
    gpsimd broadcasts 1/den; DVE multiplies into otc bf16.
  out[n,1024]   = OTcomb^T @ Wo_slice  -> fp16 partial output, DMA to DRAM

Schedule: coarse blocks in program order (Tile's priority scheduler
interleaves engines better than manual fine-grain order); batch-1
projections hide under batch-0 attention; no projection matmul sits at a
tile boundary ahead of ready attention work (PE queue head-of-line);
PSUM: 2x st(2 banks) + 3x ot/fp + 1x proj = 8 banks exactly.
"""

import numpy as np
import ml_dtypes

import concourse.bass as bass
import concourse.mybir as mybir
import concourse.tile as tile
from concourse import bacc
from concourse.bass_utils import run_bass_kernel_spmd

BF16 = mybir.dt.bfloat16
F32 = mybir.dt.float32
F16 = mybir.dt.float16
NPBF16 = ml_dtypes.bfloat16

B, N, C = 2, 2048, 1024
M = 2048          # context length
CTX = 768
H = 16
DH = 64
NCORES = 8
HEADS_PER_CORE = H // NCORES          # 2
HC = HEADS_PER_CORE * DH              # 128 columns per core
ROWS = B * N                          # 4096 flattened rows
SCALE = DH ** -0.5                    # 0.125

KQ = C // 128                         # 8 k-tiles for q projection
KC = CTX // 128                       # 6 k-tiles for kv projections
NT = 512                              # query-tile width (free dim)
MT = 128                              # m-tile (context block = partitions)
N_NT = N // NT                        # 4 query tiles per batch
N_MT = M // MT                        # 16 m-tiles per batch

_CACHE = {}


def build_kernel(repeat=1):
    """Build and compile the per-core Bass module (same program, all cores)."""
    nc = bacc.Bacc(None)

    # Host pre-tiled layouts: every DMA below is contiguous on both the
    # DRAM and SBUF side (strided 3D rearranges cost ~1-2.3us of HWDGE
    # descriptor generation per issue on the sync sequencer).
    xT_d = nc.dram_tensor("xT", [ROWS // NT, 128, KQ, NT], BF16,
                          kind="ExternalInput")
    cT_d = nc.dram_tensor("ctxT", [ROWS // NT, 128, KC, NT], BF16,
                          kind="ExternalInput")
    wq_d = nc.dram_tensor("wq", [128, KQ, HC], BF16, kind="ExternalInput")
    wk_d = nc.dram_tensor("wk", [128, KC, HC], BF16, kind="ExternalInput")
    wv_d = nc.dram_tensor("wv", [128, KC, HC], BF16, kind="ExternalInput")
    wo_d = nc.dram_tensor("wo", [HC, C], BF16, kind="ExternalInput")
    out_d = nc.dram_tensor("out", [ROWS, C], F16, kind="ExternalOutput")

    with tile.TileContext(nc) as tc:
        with (
            tc.tile_pool(name="const", bufs=1) as const,
            tc.tile_pool(name="ctx_res", bufs=1) as ctx_res,
            tc.tile_pool(name="act_res", bufs=1) as act_res,
            tc.tile_pool(name="xstream", bufs=4) as xstream,
            tc.tile_pool(name="expp", bufs=6) as expp,
            tc.tile_pool(name="otcomb", bufs=2) as otcomb_p,
            tc.tile_pool(name="nrm", bufs=4) as nrm,
            tc.tile_pool(name="ostage", bufs=3) as ostage,
            tc.tile_pool(name="pst", bufs=2, space="PSUM") as pst,
            tc.tile_pool(name="pot", bufs=3, space="PSUM") as pot,
            tc.tile_pool(name="pq", bufs=1, space="PSUM") as pq,
        ):
          for _rep in range(repeat):
              # ---- weight tiles (DMAs deferred into the startup sequence
              # so the kv-gating transfers lead the sync queue) ----
              wq_sb = const.tile([128, KQ, HC], BF16, tag="wq")
              wk_sb = const.tile([128, KC, HC], BF16, tag="wk")
              wv_sb = const.tile([128, KC, HC], BF16, tag="wv")
              wo_sb = const.tile([128, C], BF16, tag="wo")

              # resident context (quarter-major); quarters loaded
              # just-in-time in the batch loop
              ctx_sb = ctx_res.tile([128, (ROWS // NT) * KC, NT], BF16, tag="ctx")

              kT_sb = act_res.tile([128, ROWS], BF16, tag="kT")
              vago = act_res.tile([128, ROWS // 128, 130], BF16, tag="vaug")
              nc.vector.memset(vago[:, :, 64], 1.0)
              nc.vector.memset(vago[:, :, 129], 1.0)
              qT_sb = act_res.tile([128, ROWS], BF16, tag="qT")

              def xs_load(n):
                  """Issue the x-slice DMA for qT chunk n (prefetch)."""
                  xs = xstream.tile([128, KQ, NT], BF16, tag="xs")
                  nc.sync.dma_start(out=xs, in_=xT_d[n])
                  return xs

              def qt_mms(n, xs):
                  """Project qT columns [n*512, (n+1)*512) from a loaded xs."""
                  psq = pq.tile([128, NT], F32, tag="pq")
                  for t in range(KQ):
                      nc.tensor.matmul(psq, wq_sb[:, t, :], xs[:, t, :],
                                       start=(t == 0), stop=(t == KQ - 1))
                  nc.vector.tensor_copy(qT_sb[:, bass.ts(n, NT)], psq)

              def ctx_quarter(n):
                  nc.sync.dma_start(out=ctx_sb[:, n * KC:(n + 1) * KC, :], in_=cT_d[n])

              def kvk(n):
                  """Project kT cols [n*512, (n+1)*512) (needs ctx quarter n)."""
                  ps = pq.tile([128, NT], F32, tag="pq")
                  for t in range(KC):
                      nc.tensor.matmul(ps, wk_sb[:, t, :],
                                       ctx_sb[:, n * KC + t, :],
                                       start=(t == 0), stop=(t == KC - 1))
                  nc.vector.tensor_copy(kT_sb[:, bass.ts(n, NT)], ps)

              def kvv(m):
                  """Project v for global m-tile m into vago."""
                  psv = pq.tile([128, NT], F32, tag="pq")
                  for t in range(KC):
                      nc.tensor.matmul(psv[:, 0:HC],
                                       ctx_sb[:, (m // 4) * KC + t, bass.ts(m % 4, 128)],
                                       wv_sb[:, t, :],
                                       start=(t == 0), stop=(t == KC - 1))
                  nc.vector.tensor_copy(vago[:, m, 0:64], psv[:, 0:64])
                  nc.vector.tensor_copy(vago[:, m, 65:129], psv[:, 64:128])

              def attn1(b, nt, mt, ot_A, ot_B, otc=None):
                  """S^T + exp + AV for one m-tile of query tile (b, nt).

                  Both heads' K=64 S^T matmuls run concurrently in the PE
                  array (row groups 0:64 / 64:128) into one 2-bank PSUM
                  tile; one ScalarE Exp call covers both heads."""
                  nsl = bass.ds(b * N + nt * NT, NT)
                  msl = bass.ds(b * M + mt * MT, MT)
                  st = pst.tile([128, 2 * NT], F32, tag="st")
                  for h in range(2):
                      hd = bass.ds(h * DH, DH)
                      nc.tensor.matmul(st[:, bass.ts(h, NT)], kT_sb[hd, msl],
                                       qT_sb[hd, nsl],
                                       start=True, stop=True,
                                       tile_position=(h * DH, 0))
                  exp_sb = expp.tile([128, 2 * NT], BF16, tag="exp")
                  nc.scalar.activation(
                      exp_sb, st,
                      mybir.ActivationFunctionType.Exp, scale=SCALE)
                  for h, ot_ps in ((0, ot_A), (1, ot_B)):
                      nc.tensor.matmul(
                          ot_ps,
                          vago[:, (b * M) // 128 + mt, bass.ds(h * 65, 65)],
                          exp_sb[:, bass.ts(h, NT)],
                          start=(mt == 0), stop=(mt == N_MT - 1))
                  if otc is not None and mt == N_MT - 1:
                      normalize_pair(ot_A, ot_B, otc)

              def normalize_pair(ot_A, ot_B, otc):
                  """softmax-normalize both heads' O^T into otc (bf16).

                  All four PSUM->SBUF evacuations run on ScalarE (idle at
                  tile boundaries) so the two pot slots free fast without
                  entering the DVE FIFO. The reciprocal of the [1,1024]
                  denominator row runs lane-parallel via the DVE 32x32
                  transpose trick (~2.7us total vs 6.6us of single-lane
                  iterative divides)."""
                  cp = mybir.ActivationFunctionType.Copy
                  osb_A = nrm.tile([64, NT], F32, tag="osbA")
                  nc.scalar.activation(osb_A, ot_A[0:64, :], cp)
                  osb_B = nrm.tile([64, NT], F32, tag="osbB")
                  nc.scalar.activation(osb_B, ot_B[0:64, :], cp)
                  dsb = nrm.tile([32, 2 * NT], F32, tag="dsb")
                  nc.scalar.activation(dsb[0:1, 0:NT], ot_A[64:65, :], cp)
                  nc.scalar.activation(dsb[0:1, NT:2 * NT], ot_B[64:65, :], cp)
                  # den row -> 32 partitions -> wide recip -> back to a row
                  t1 = nrm.tile([32, 2 * NT], F32, tag="t1")
                  nc.vector.transpose(t1, dsb)
                  nc.vector.reciprocal(t1[:, 0:2 * NT:32], t1[:, 0:2 * NT:32])
                  t2 = nrm.tile([32, 2 * NT], F32, tag="t2")
                  nc.vector.transpose(t2, t1)
                  for h, osb in ((0, osb_A), (1, osb_B)):
                      bc = nrm.tile([64, NT], F32, tag="bc")
                      nc.gpsimd.partition_broadcast(bc, t2[0:1, bass.ts(h, NT)])
                      if h == 0:
                          nc.vector.tensor_mul(otc[0:64, :], osb, bc)
                      else:
                          otn = nrm.tile([64, NT], BF16, tag="otn")
                          nc.vector.tensor_mul(otn, osb, bc)
                          # partition shift 0:64 -> 64:128 via SBUF->SBUF DMA
                          nc.sync.dma_start(out=otc[64:128, :], in_=otn)

              def fo_piece(b, nt, otc, s):
                  """One 128-row slice of out = otc^T @ Wo."""
                  ost = ostage.tile([128, C], F16, tag="ost")
                  for cpart in range(C // NT):
                      fp = pot.tile([128, NT], F32, tag="ot")
                      nc.tensor.matmul(fp, otc[:, bass.ts(s, 128)],
                                       wo_sb[:, bass.ts(cpart, NT)],
                                       start=True, stop=True)
                      nc.vector.tensor_copy(ost[:, bass.ts(cpart, NT)], fp)
                  nc.sync.dma_start(
                      out=out_d[bass.ds(b * N + nt * NT + s * 128, 128), :],
                      in_=ost)

              def final_out(b, nt, otc):
                  for s in range(NT // 128):
                      fo_piece(b, nt, otc, s)

              # ---- orchestration (coarse blocks — the Tile scheduler
              # interleaves engines better than manual fine-grain order,
              # which measurably inflates instruction durations) ----
              def attn_mtiles(b, nt, ot_A, ot_B, mts, otc=None):
                  for mt in mts:
                      attn1(b, nt, mt, ot_A, ot_B, otc=otc)

              def kv_chunk(n):
                  kvk(n)
                  for mm in range(4):
                      kvv(n * 4 + mm)

              def qt_chunk(n):
                  qt_mms(n, xs_load(n))

              pending = None      # (b, nt, otc) awaiting final projection
              # batch 0 tile 0: startup — DMA order puts the kv-chain
              # gates (ctx quarter 0, wk, wv) ahead of the q-chain.
              ctx_quarter(0)
              nc.sync.dma_start(out=wk_sb, in_=wk_d[:])
              nc.sync.dma_start(out=wv_sb, in_=wv_d[:])
              nc.sync.dma_start(out=wq_sb, in_=wq_d[:])
              kv_chunk(0)
              qt_chunk(0)
              ot_A = pot.tile([65, NT], F32, tag="ot")
              ot_B = pot.tile([65, NT], F32, tag="ot")
              otc = otcomb_p.tile([128, NT], BF16, tag="otc")
              attn_mtiles(0, 0, ot_A, ot_B, range(0, 4), otc=otc)
              for c in range(1, 4):
                  ctx_quarter(c)
                  if c == 1:
                      nc.sync.dma_start(out=wo_sb, in_=wo_d[:])
                  kv_chunk(c)
                  qt_chunk(c)
                  attn_mtiles(0, 0, ot_A, ot_B, range(4 * c, 4 * c + 4), otc=otc)
              pending = (0, 0, otc)
              # batch 0 tiles 1-3, with batch 1's projections hidden under
              # their ACT-bound attention; qt chunks placed just after a
              # tile's first attention block so no projection matmul sits
              # at a tile boundary ahead of ready attention work.
              for nt in range(1, N_NT):
                  ot_A = pot.tile([65, NT], F32, tag="ot")
                  ot_B = pot.tile([65, NT], F32, tag="ot")
                  otc = otcomb_p.tile([128, NT], BF16, tag="otc")
                  attn_mtiles(0, nt, ot_A, ot_B, range(0, 4))
                  if nt >= 2:
                      qt_chunk(N_NT + nt - 2)
                  c = nt - 1            # b1 chunks 0..2 under b0 tiles 1..3
                  ctx_quarter(4 + c)
                  kv_chunk(4 + c)
                  attn_mtiles(0, nt, ot_A, ot_B, range(4, 8))
                  final_out(*pending)
                  attn_mtiles(0, nt, ot_A, ot_B, range(8, N_MT), otc=otc)
                  pending = (0, nt, otc)
              # batch 1
              for nt in range(N_NT):
                  ot_A = pot.tile([65, NT], F32, tag="ot")
                  ot_B = pot.tile([65, NT], F32, tag="ot")
                  otc = otcomb_p.tile([128, NT], BF16, tag="otc")
                  attn_mtiles(1, nt, ot_A, ot_B, range(0, 4))
                  if nt == 0:
                      qt_chunk(N_NT + 2)
                      ctx_quarter(7)
                      kv_chunk(7)
                      attn_mtiles(1, 0, ot_A, ot_B, range(4, 8))
                      final_out(*pending)
                      attn_mtiles(1, 0, ot_A, ot_B, range(8, N_MT), otc=otc)
                  else:
                      if nt == 1:
                          qt_chunk(N_NT + 3)
                      attn_mtiles(1, nt, ot_A, ot_B, range(4, 8))
                      final_out(*pending)
                      attn_mtiles(1, nt, ot_A, ot_B, range(8, N_MT), otc=otc)
                  pending = (1, nt, otc)
              final_out(*pending)

    nc.compile()
    return nc


def _shard_inputs(x, context, Wq, Wkv, Wo):
    # Pre-tile on host so every device DMA is contiguous on both sides:
    #   xT[n, p, t, j]  = x^T[t*128+p, n*512+j]   (chunk-major)
    #   ctxT[q, p, t, j] = ctx^T[t*128+p, q*512+j] (quarter-major)
    #   wq/wk/wv[p, t, m] = W[t*128+p, m]
    xf = x.reshape(ROWS, C).T.astype(NPBF16)                 # [C, ROWS]
    x8 = np.ascontiguousarray(
        xf.reshape(KQ, 128, ROWS // NT, NT).transpose(2, 1, 0, 3))
    cf = context.reshape(ROWS, CTX).T.astype(NPBF16)         # [CTX, ROWS]
    c8 = np.ascontiguousarray(
        cf.reshape(KC, 128, ROWS // NT, NT).transpose(2, 1, 0, 3))
    in_maps = []
    for c in range(NCORES):
        hc = slice(HC * c, HC * (c + 1))
        wq = np.ascontiguousarray(
            Wq[:, hc].astype(NPBF16).reshape(KQ, 128, HC).transpose(1, 0, 2))
        wk = np.ascontiguousarray(
            Wkv[:, hc].astype(NPBF16).reshape(KC, 128, HC).transpose(1, 0, 2))
        wv = np.ascontiguousarray(
            Wkv[:, C + HC * c:C + HC * (c + 1)].astype(NPBF16)
            .reshape(KC, 128, HC).transpose(1, 0, 2))
        in_maps.append({
            "xT": x8,
            "ctxT": c8,
            "wq": wq,
            "wk": wk,
            "wv": wv,
            "wo": np.ascontiguousarray(Wo[hc, :]).astype(NPBF16),
        })
    return in_maps


def get_nc():
    if "nc" not in _CACHE:
        _CACHE["nc"] = build_kernel()
    return _CACHE["nc"]


def run_cores(in_maps, **kw):
    nc = get_nc()
    return run_bass_kernel_spmd(nc, in_maps, list(range(NCORES)), **kw)


def kernel(x, context, Wq, Wkv, Wo):
    in_maps = _shard_inputs(
        np.asarray(x, np.float32), np.asarray(context, np.float32),
        np.asarray(Wq, np.float32), np.asarray(Wkv, np.float32),
        np.asarray(Wo, np.float32))
    res = run_cores(in_maps)
    acc = res.results[0]["out"].astype(np.float32)
    for i in range(1, NCORES):
        acc = acc + res.results[i]["out"].astype(np.float32)
    return acc.reshape(B, N, C)
